# revision 17
# baseline (speedup 1.0000x reference)
"""Trainium2 Bass kernel for nn_ColorHistograms.

Pipeline (per NeuronCore, 2 batch elements each, 8 cores):
  1. Stream x tiles [128 frames, 3888] from HBM in 3 chunk-DMAs each (spreads
     load over DMA engines; one engine tops out at ~22.5 GB/s). Per-channel
     spatial means: channels 0,1 on ScalarE (activation Copy + accum_out),
     channel 2 on VectorE (strided tensor_reduce).
  2. PE-transpose the [128, 24] mean columns; stage them to a DRAM scratch as
     fp16 centered at 0.5 (mean-of-1296-uniforms is 0.5 +- 0.01, and only
     differences matter downstream, so fp16 keeps ~1e-3 relative accuracy
     while halving gather traffic).
  3. Toeplitz gather DMAs (one per channel) materialize all 101 shifted
     copies of the padded mean rows; a 0-stride gather broadcasts the base.
  4. VectorE: fp16 in-place diff, |.| channel-reduce to fp32, out-of-range
     mask multiply -> window features [101, 1024] (+ ones row = bias trick).
  5. PE matmul [102]x[128 t]x[128 out] with fc weights (bias folded in as an
     extra contraction row), VectorE relu PSUM->SBUF, contiguous DMA out.
"""

import sys

if "/opt/trn_rl_repo" not in sys.path:
    sys.path.insert(0, "/opt/trn_rl_repo")

import numpy as np

N_CORES = 8
B, T, H, W, C = 16, 1024, 27, 48, 3
S = H * W                 # 1296 spatial positions
ROW = S * C               # 3888 floats per frame
LW = 101                  # lookup window
PAD = 50
OD = 128                  # output dim
BPC = B // N_CORES        # batches per core = 2
PADROW = T + LW - 1       # 1124
FT = T // 128             # 8 frame-tiles per batch
XCH = 2                   # chunk-DMAs per x tile
CENTER = 0.5              # mean centering before the fp16 staging cast
# The runtime picks the DMA engine from the DRAM-side address granule, so a
# gather whose reads all land in one ~13 KB region serializes onto a single
# engine (~24 GB/s). Stage the mean rows into NREP replicas spaced REPS
# elements apart and split each gather into w-chunks reading distinct
# replicas so the chunks land on distinct engines. The stride is an odd
# multiple of 2/4/8 KB so replicas stay distinct mod 16 for any granule size.
NREP = 4
REPS = 70656              # fp16 elements between replicas (141312 B)
WSPLIT = [0, 26, 52, 78, LW]

_CACHE = {}


def _build_program():
    import concourse.bass as bass
    import concourse.tile as tile
    from concourse import bacc, mybir
    from concourse.ap import AP

    f32 = mybir.dt.float32
    f16 = mybir.dt.float16
    bf16 = mybir.dt.bfloat16
    nc = bacc.Bacc("TRN2", target_bir_lowering=False, debug=False)

    xs = nc.dram_tensor("xs", [BPC * T, ROW], f32, kind="ExternalInput")
    fcwb = nc.dram_tensor("fcwb", [LW + 1, OD], f32, kind="ExternalInput")
    maskw = nc.dram_tensor("maskw", [LW, T], bf16, kind="ExternalInput")
    ident = nc.dram_tensor("ident", [128, 128], f32, kind="ExternalInput")
    y = nc.dram_tensor("y", [BPC * T, OD], f32, kind="ExternalOutput")
    mcpad = nc.dram_tensor("mcpad", [NREP * REPS], f16)
    mc_ap = mcpad[:]

    def mc_view(offset, dims):
        return AP(tensor=mc_ap.tensor, offset=offset, ap=tuple(dims))

    with tile.TileContext(nc) as tc:
        with (
            tc.tile_pool(name="consts", bufs=1) as consts,
            tc.tile_pool(name="xin", bufs=5) as xin,
            tc.tile_pool(name="junk", bufs=1) as junkp,
            tc.tile_pool(name="sums", bufs=2) as sumsp,
            tc.tile_pool(name="stg", bufs=2) as stgp,
            tc.tile_pool(name="gath", bufs=2) as gathp,
            tc.tile_pool(name="wf", bufs=2) as wfp,
            tc.tile_pool(name="outs", bufs=4) as outsp,
            tc.tile_pool(name="zrow", bufs=1) as zrowp,
            tc.tile_pool(name="pst", bufs=2, space="PSUM") as pst,
            tc.tile_pool(name="pso", bufs=4, space="PSUM") as pso,
        ):
            fcwb_sb = consts.tile([LW + 1, OD], f32)
            nc.gpsimd.dma_start(fcwb_sb[:], fcwb[:])
            maskw_sb = consts.tile([LW, T], bf16)
            nc.gpsimd.dma_start(maskw_sb[:], maskw[:])
            ident_sb = consts.tile([128, 128], f32)
            nc.gpsimd.dma_start(ident_sb[:], ident[:])

            # zero-fill the used region of each replica (the padded wings
            # must read as 0.0; the inter-replica gaps are never read)
            z = zrowp.tile([24, PADROW], f16)
            nc.vector.memset(z[:], 0.0)
            nc.gpsimd.dma_start(
                mc_view(0, [(REPS, NREP), (1, BPC * C * PADROW)]), z[:]
            )

            for b in range(BPC):
                # -------- stage A: spatial means --------
                sums = sumsp.tile([128, C * FT], f32)
                for i in range(FT):
                    xt = xin.tile([128, ROW], f32)
                    # split by partition halves: same bytes, half the
                    # descriptor-generation cost on the sync HWDGE ring
                    for k in range(XCH):
                        p0, p1 = k * (128 // XCH), (k + 1) * (128 // XCH)
                        nc.sync.dma_start(
                            xt[p0:p1, :],
                            xs[b * T + i * 128 + p0 : b * T + i * 128 + p1, :],
                        )
                    xv = xt[:].rearrange("p (s c) -> p c s", c=C)
                    junk = junkp.tile([128, S], f32)
                    for c in range(2):
                        nc.scalar.activation(
                            junk[:],
                            xv[:, c, :],
                            mybir.ActivationFunctionType.Copy,
                            bias=0.0,
                            scale=1.0 / S,
                            accum_out=sums[:, c * FT + i : c * FT + i + 1],
                        )
                    # channel 2 on VectorE; fold the 1/S scale into the PE
                    # transpose? no - scale applied at staging would also
                    # scale the center; just reduce raw and scale below.
                    nc.vector.tensor_reduce(
                        sums[:, 2 * FT + i : 2 * FT + i + 1],
                        xv[:, 2, :],
                        axis=mybir.AxisListType.X,
                        op=mybir.AluOpType.add,
                    )
                # channel-2 columns hold raw sums (VectorE reduce has no
                # scale); normalize them before the transpose so the staging
                # cast below is one op over all 24 partitions.
                nc.vector.tensor_scalar_mul(
                    sums[:, 2 * FT : C * FT], sums[:, 2 * FT : C * FT], 1.0 / S
                )
                # transpose [128, 24] -> [24, 128], then center+cast to fp16
                ps = pst.tile([C * FT, 128], f32)
                nc.tensor.transpose(ps[:], sums[:], ident_sb[:])
                stg = stgp.tile([C * FT, 128], f16)
                nc.vector.tensor_scalar_sub(stg[:], ps[:], CENTER)
                for r in range(NREP):
                    dst = mc_view(
                        r * REPS + b * C * PADROW + PAD,
                        [(PADROW, C), (128, FT), (1, 128)],
                    )
                    nc.gpsimd.dma_start(dst, stg[:])

                # -------- stage B: windowed band L1 distances --------
                # gathers go on the scalar engine's separate HWDGE ring so
                # they never contend with the x-stream on the sync ring
                sh = gathp.tile([LW, C * T], f16)
                ba = gathp.tile([LW, C * T], f16)
                for k in range(4):
                    w0, w1 = WSPLIT[k], WSPLIT[k + 1]
                    nc.scalar.dma_start(
                        sh[w0:w1, :],
                        mc_view(
                            k * REPS + b * C * PADROW + w0,
                            [(1, w1 - w0), (PADROW, C), (1, T)],
                        ),
                    )
                # base row (= mc centered, in-range) broadcast on-chip:
                # 6 KB row DMA + gpsimd partition replication
                row = stgp.tile([1, C * T], f16, tag="barow")
                nc.gpsimd.dma_start(
                    row[:],
                    mc_view(b * C * PADROW + PAD, [(1, 1), (PADROW, C), (1, T)]),
                )
                nc.gpsimd.partition_broadcast(ba[:], row[:], channels=LW)

                wf = wfp.tile([LW + 1, T], f32)
                # row LW must be 1.0 (bias trick); engines can only start at
                # partition 0/32/64/96, so fill everything and overwrite 0..100
                nc.vector.memset(wf[:], 1.0)
                nc.vector.tensor_sub(sh[:], sh[:], ba[:])
                shv = sh[:].rearrange("p (c t) -> p t c", c=C)
                nc.vector.tensor_reduce(
                    wf[0:LW, :],
                    shv,
                    axis=mybir.AxisListType.X,
                    op=mybir.AluOpType.add,
                    apply_absolute_value=True,
                )
                nc.vector.tensor_mul(wf[0:LW, :], wf[0:LW, :], maskw_sb[:])

                # -------- stage C: linear + relu --------
                for j in range(FT):
                    po = pso.tile([128, OD], f32)
                    nc.tensor.matmul(
                        po[:], wf[:, bass.ts(j, 128)], fcwb_sb[:]
                    )
                    osb = outsp.tile([128, OD], f32)
                    nc.vector.tensor_scalar_max(osb[:], po[:], 0.0)
                    # batch 0 outputs go mid-stream on the idle gpsimd ring;
                    # batch 1 outputs are on the tail - split them across the
                    # sync+scalar rings (both idle by then) to halve issue time
                    if b == 0:
                        eng = nc.gpsimd
                    else:
                        eng = nc.sync if j % 2 == 0 else nc.scalar
                    eng.dma_start(
                        y[b * T + j * 128 : b * T + (j + 1) * 128, :], osb[:]
                    )

    nc.compile()
    return nc


def get_nc():
    if "nc" not in _CACHE:
        _CACHE["nc"] = _build_program()
    return _CACHE["nc"]


def make_host_inputs(x, fc_w, fc_b):
    """Per-core input maps from the full problem inputs."""
    x = np.ascontiguousarray(x, dtype=np.float32).reshape(B, T, ROW)
    fcwb = np.concatenate(
        [fc_w.T.astype(np.float32), fc_b[None, :].astype(np.float32)], axis=0
    )
    fcwb = np.ascontiguousarray(fcwb)
    u = np.arange(T)[None, :] + np.arange(LW)[:, None] - PAD
    import ml_dtypes

    maskw = ((u >= 0) & (u < T)).astype(ml_dtypes.bfloat16)
    ident = np.eye(128, dtype=np.float32)
    in_maps = []
    for ci in range(N_CORES):
        shard = np.ascontiguousarray(
            x[ci * BPC : (ci + 1) * BPC].reshape(BPC * T, ROW)
        )
        in_maps.append(
            {"xs": shard, "fcwb": fcwb, "maskw": maskw, "ident": ident}
        )
    return in_maps


def kernel(x, fc_w, fc_b):
    from concourse.bass_utils import run_bass_kernel_spmd

    nc = get_nc()
    in_maps = make_host_inputs(x, fc_w, fc_b)
    res = run_bass_kernel_spmd(nc, in_maps, list(range(N_CORES)))
    outs = [r["y"].reshape(BPC, T, OD) for r in res.results]
    return np.concatenate(outs, axis=0).astype(np.float32)


# revision 21
# speedup vs baseline: 1.5932x; 1.5932x over previous
"""Trainium2 Bass kernel for nn_ColorHistograms.

Pipeline (per NeuronCore, 2 batch elements each, 8 cores):
  1. Stream x tiles [128 frames, 3888] from HBM in 3 chunk-DMAs each (spreads
     load over DMA engines; one engine tops out at ~22.5 GB/s). Per-channel
     spatial means: channels 0,1 on ScalarE (activation Copy + accum_out),
     channel 2 on VectorE (strided tensor_reduce).
  2. PE-transpose the [128, 24] mean columns; stage them to a DRAM scratch as
     fp16 centered at 0.5 (mean-of-1296-uniforms is 0.5 +- 0.01, and only
     differences matter downstream, so fp16 keeps ~1e-3 relative accuracy
     while halving gather traffic).
  3. Toeplitz gather DMAs (one per channel) materialize all 101 shifted
     copies of the padded mean rows; a 0-stride gather broadcasts the base.
  4. VectorE: fp16 in-place diff, |.| channel-reduce to fp32, out-of-range
     mask multiply -> window features [101, 1024] (+ ones row = bias trick).
  5. PE matmul [102]x[128 t]x[128 out] with fc weights (bias folded in as an
     extra contraction row), VectorE relu PSUM->SBUF, contiguous DMA out.
"""

import sys

if "/opt/trn_rl_repo" not in sys.path:
    sys.path.insert(0, "/opt/trn_rl_repo")

import numpy as np

N_CORES = 8
B, T, H, W, C = 16, 1024, 27, 48, 3
S = H * W                 # 1296 spatial positions
ROW = S * C               # 3888 floats per frame
LW = 101                  # lookup window
PAD = 50
OD = 128                  # output dim
BPC = B // N_CORES        # batches per core = 2
PADROW = T + LW - 1       # 1124
FT = T // 128             # 8 frame-tiles per batch
XCH = 2                   # chunk-DMAs per x tile
CENTER = 0.5              # mean centering before the fp16 staging cast
# The runtime picks the DMA engine from the DRAM-side address granule, so a
# gather whose reads all land in one ~13 KB region serializes onto a single
# engine (~24 GB/s). Stage the mean rows into NREP replicas spaced REPS
# elements apart and split each gather into w-chunks reading distinct
# replicas so the chunks land on distinct engines. The stride is an odd
# multiple of 2/4/8 KB so replicas stay distinct mod 16 for any granule size.
NREP = 4
REPS = 70656              # fp16 elements between replicas (141312 B)
# window rows are PERMUTED so that dest row 0 is w=50 (the base row): the
# on-chip base broadcast can then read partition 0 of the gather tile with no
# extra row DMA. fc weights and the mask are row-permuted on the host to
# match, so the matmul contraction is unchanged.
PERM = [50] + [w for w in range(LW) if w != 50]
# (dest_row0, dest_row1, src_w0, replica)
WCHUNKS = [(0, 1, 50, 3), (1, 26, 0, 0), (26, 51, 25, 1),
           (51, 76, 51, 2), (76, 101, 76, 3)]

_CACHE = {}


def _build_program():
    import concourse.bass as bass
    import concourse.tile as tile
    from concourse import bacc, mybir
    from concourse.ap import AP

    f32 = mybir.dt.float32
    f16 = mybir.dt.float16
    bf16 = mybir.dt.bfloat16
    nc = bacc.Bacc("TRN2", target_bir_lowering=False, debug=False)

    xs = nc.dram_tensor("xs", [BPC * T, ROW], f32, kind="ExternalInput")
    fcwb = nc.dram_tensor("fcwb", [LW + 1, OD], f32, kind="ExternalInput")
    maskw = nc.dram_tensor("maskw", [LW, T], bf16, kind="ExternalInput")
    ident = nc.dram_tensor("ident", [128, 128], f32, kind="ExternalInput")
    y = nc.dram_tensor("y", [BPC * T, OD], f32, kind="ExternalOutput")
    mcpad = nc.dram_tensor("mcpad", [NREP * REPS], f16)
    mc_ap = mcpad[:]

    def mc_view(offset, dims):
        return AP(tensor=mc_ap.tensor, offset=offset, ap=tuple(dims))

    with tile.TileContext(nc) as tc:
        with (
            tc.tile_pool(name="consts", bufs=1) as consts,
            tc.tile_pool(name="xin", bufs=5) as xin,
            tc.tile_pool(name="junk", bufs=1) as junkp,
            tc.tile_pool(name="sums", bufs=2) as sumsp,
            tc.tile_pool(name="stg", bufs=2) as stgp,
            tc.tile_pool(name="gath", bufs=2) as gathp,
            tc.tile_pool(name="wf", bufs=2) as wfp,
            tc.tile_pool(name="outs", bufs=4) as outsp,
            tc.tile_pool(name="zrow", bufs=1) as zrowp,
            tc.tile_pool(name="pst", bufs=2, space="PSUM") as pst,
            tc.tile_pool(name="pso", bufs=4, space="PSUM") as pso,
        ):
            fcwb_sb = consts.tile([LW + 1, OD], f32)
            nc.gpsimd.dma_start(fcwb_sb[:], fcwb[:])
            maskw_sb = consts.tile([LW, T], bf16)
            nc.gpsimd.dma_start(maskw_sb[:], maskw[:])
            ident_sb = consts.tile([128, 128], f32)
            nc.gpsimd.dma_start(ident_sb[:], ident[:])

            # zero-fill the used region of each replica (the padded wings
            # must read as 0.0; the inter-replica gaps are never read)
            z = zrowp.tile([24, PADROW], f16)
            nc.vector.memset(z[:], 0.0)
            nc.gpsimd.dma_start(
                mc_view(0, [(REPS, NREP), (1, BPC * C * PADROW)]), z[:]
            )

            for b in range(BPC):
                # -------- stage A: spatial means --------
                sums = sumsp.tile([128, C * FT], f32)
                for i in range(FT):
                    xt = xin.tile([128, ROW], f32)
                    # split along the free dim: partition splits halve the
                    # per-engine DMA bandwidth (AXI ports map to partitions)
                    for k in range(XCH):
                        lo, hi = k * (ROW // XCH), (k + 1) * (ROW // XCH)
                        nc.sync.dma_start(
                            xt[:, lo:hi],
                            xs[b * T + i * 128 : b * T + (i + 1) * 128, lo:hi],
                        )
                    xv = xt[:].rearrange("p (s c) -> p c s", c=C)
                    junk = junkp.tile([128, S], f32)
                    for c in range(2):
                        nc.scalar.activation(
                            junk[:],
                            xv[:, c, :],
                            mybir.ActivationFunctionType.Copy,
                            bias=0.0,
                            scale=1.0 / S,
                            accum_out=sums[:, c * FT + i : c * FT + i + 1],
                        )
                    # channel 2 on VectorE; fold the 1/S scale into the PE
                    # transpose? no - scale applied at staging would also
                    # scale the center; just reduce raw and scale below.
                    nc.vector.tensor_reduce(
                        sums[:, 2 * FT + i : 2 * FT + i + 1],
                        xv[:, 2, :],
                        axis=mybir.AxisListType.X,
                        op=mybir.AluOpType.add,
                    )
                # channel-2 columns hold raw sums (VectorE reduce has no
                # scale); normalize them before the transpose so the staging
                # cast below is one op over all 24 partitions.
                nc.vector.tensor_scalar_mul(
                    sums[:, 2 * FT : C * FT], sums[:, 2 * FT : C * FT], 1.0 / S
                )
                # transpose [128, 24] -> [24, 128], then center+cast to fp16
                ps = pst.tile([C * FT, 128], f32)
                nc.tensor.transpose(ps[:], sums[:], ident_sb[:])
                stg = stgp.tile([C * FT, 128], f16)
                nc.vector.tensor_scalar_sub(stg[:], ps[:], CENTER)
                for r in range(NREP):
                    dst = mc_view(
                        r * REPS + b * C * PADROW + PAD,
                        [(PADROW, C), (128, FT), (1, 128)],
                    )
                    nc.gpsimd.dma_start(dst, stg[:])

                # -------- stage B: windowed band L1 distances --------
                # gathers go on the scalar engine's separate HWDGE ring so
                # they never contend with the x-stream on the sync ring
                sh = gathp.tile([LW, C * T], f16)
                ba = gathp.tile([LW, C * T], f16)
                for r0, r1, w0, rep in WCHUNKS:
                    nc.scalar.dma_start(
                        sh[r0:r1, :],
                        mc_view(
                            rep * REPS + b * C * PADROW + w0,
                            [(1, r1 - r0), (PADROW, C), (1, T)],
                        ),
                    )
                # base row (= mc centered, in-range) = permuted row 0 of sh;
                # replicate it across partitions on the idle gpsimd engine
                nc.gpsimd.partition_broadcast(ba[:], sh[0:1, :], channels=LW)

                wf = wfp.tile([LW + 1, T], f32)
                # row LW must be 1.0 (bias trick); engines can only start at
                # partition 0/32/64/96, so fill everything and overwrite 0..100
                nc.vector.memset(wf[:], 1.0)
                nc.vector.tensor_sub(sh[:], sh[:], ba[:])
                shv = sh[:].rearrange("p (c t) -> p t c", c=C)
                nc.vector.tensor_reduce(
                    wf[0:LW, :],
                    shv,
                    axis=mybir.AxisListType.X,
                    op=mybir.AluOpType.add,
                    apply_absolute_value=True,
                )
                nc.vector.tensor_mul(wf[0:LW, :], wf[0:LW, :], maskw_sb[:])

                # -------- stage C: linear + relu --------
                for j in range(FT):
                    po = pso.tile([128, OD], f32)
                    nc.tensor.matmul(
                        po[:], wf[:, bass.ts(j, 128)], fcwb_sb[:]
                    )
                    osb = outsp.tile([128, OD], f32)
                    nc.vector.tensor_scalar_max(osb[:], po[:], 0.0)
                    # batch 0 outputs go mid-stream on the idle gpsimd ring;
                    # batch 1 outputs are on the tail - split them across the
                    # sync+scalar rings (both idle by then) to halve issue time
                    if b == 0:
                        eng = nc.gpsimd
                    else:
                        eng = nc.sync if j % 2 == 0 else nc.scalar
                    eng.dma_start(
                        y[b * T + j * 128 : b * T + (j + 1) * 128, :], osb[:]
                    )

    nc.compile()
    return nc


def get_nc():
    if "nc" not in _CACHE:
        _CACHE["nc"] = _build_program()
    return _CACHE["nc"]


def make_host_inputs(x, fc_w, fc_b):
    """Per-core input maps from the full problem inputs."""
    x = np.ascontiguousarray(x, dtype=np.float32).reshape(B, T, ROW)
    wT = fc_w.T.astype(np.float32)[PERM]          # window-row permutation
    fcwb = np.concatenate([wT, fc_b[None, :].astype(np.float32)], axis=0)
    fcwb = np.ascontiguousarray(fcwb)
    u = np.arange(T)[None, :] + np.arange(LW)[:, None] - PAD
    import ml_dtypes

    maskw = ((u >= 0) & (u < T)).astype(ml_dtypes.bfloat16)[PERM]
    maskw = np.ascontiguousarray(maskw)
    ident = np.eye(128, dtype=np.float32)
    in_maps = []
    for ci in range(N_CORES):
        shard = np.ascontiguousarray(
            x[ci * BPC : (ci + 1) * BPC].reshape(BPC * T, ROW)
        )
        in_maps.append(
            {"xs": shard, "fcwb": fcwb, "maskw": maskw, "ident": ident}
        )
    return in_maps


def kernel(x, fc_w, fc_b):
    from concourse.bass_utils import run_bass_kernel_spmd

    nc = get_nc()
    in_maps = make_host_inputs(x, fc_w, fc_b)
    res = run_bass_kernel_spmd(nc, in_maps, list(range(N_CORES)))
    outs = [r["y"].reshape(BPC, T, OD) for r in res.results]
    return np.concatenate(outs, axis=0).astype(np.float32)
